# revision 29
# baseline (speedup 1.0000x reference)
"""Trainium2 Bass kernel for nn_Classifier_22299470201420 (retrieval_knn).

Reference computation:
    hv   = (samples - 0.5) @ W.T          # [B, D] random projection
    bip  = where(hv > 0, 1, -1)           # bipolar hypervector
    dots = bip @ (2*centroids - 1).T      # [B, C] bipolar dot products
    sim  = int32(0.5 * (D + dots))        # hamming similarity counts

Sharding: tensor-parallel over the D (dimensions) axis — D is zero-padded
10000 -> 10240 = 80 d-tiles of 128; each of the 8 cores owns 10 d-tiles
and the FULL batch, producing a partial dots [C, B] that the host sums
(exact integer adds, no on-device collective needed).

Precision: samples-0.5 is quantized to fp8e4m3 only (no lo-residual
pass).  W is {-1,+1} — exact in fp8 — and hv accumulates in fp32 PSUM,
so the only error is the fp8 input quantization, which flips the sign
of hv for ~0.8% of bits (only where |hv| is tiny): measured host-side
rel err 1.73e-3 on the output counts, far below the 2e-2 gate.

Device kernel (per core), all work inside the timed body:
  - 8 b-chunks of 512 samples; per chunk 10 d-tiles of 4 fp8 DoubleRow
    matmuls (K=256 each) accumulate hv^T into PSUM.  The PE is the
    critical resource (HW-measured ~274ns per 512-row DR matmul,
    weight loads fully hidden); everything else is sized to stay under
    it so the PE never stalls.
  - binarize hv^T: one instruction per d-tile pair over a 2-bank PSUM
    tile, split across ScalarE (Sign -> bipolar) and DVE (is_gt ->
    binary {1,0}); binary tiles use doubled centroids (exact fp8) in
    matmul2 and the host subtracts the per-class sum of cb over binary
    tiles — exact.  Each mm2 is deferred 4 mm1-groups in the PE stream
    so the in-order PE never waits on the sign engines.
  - matmul2 runs fp8 DoubleRow over d-tile pairs, accumulating into one
    PSUM bank [112, 512] per chunk; DVE copies it to SBUF and a Pool
    SWDGE DMA writes it out (GPSIMD cannot read PSUM on real TRN2).
  - head-latency: input DMA configs are spread over SP/Pool/Act so no
    single sequencer serializes the head, and a dummy 1-element Sign
    preloads the ScalarE activation table (1.3us) during the DMA head
    instead of stalling the first mm2.
  - The final affine 0.5*(D + dots - binsum) + int32 cast + transpose
    happens on the host on the [C, B] summed partials.
"""

import os

import numpy as np
import ml_dtypes

B, F, D, C = 4096, 1024, 10000, 100
NCORES = 8
NT = 80                   # global 128-wide d-tiles (D padded 10000 -> 10240)
NTL = NT // NCORES        # d-tiles per core
DPAD = NT * 128           # 10240
FG = F // 128             # 8 f-groups of 128
NB = 8                    # b-chunks per core (full batch)
BCK = B // NB             # 512
CP = 112                  # C padded so fp8 DoubleRow weight strides are 16B-aligned
NWA = 5                   # d-tiles in the first weight DMA (needed earliest)

bf16 = ml_dtypes.bfloat16
f8 = ml_dtypes.float8_e4m3

# engine per local d-tile pair: Act pairs produce bipolar {-1,+1} via
# Sign; DVE pairs produce binary {1,0} via is_gt and use doubled
# centroids with an exact host-side correction.  Each pair is binarized
# by ONE instruction over a 2-bank PSUM tile [128, 2, 512].  (Pool/
# GPSIMD cannot read PSUM on real TRN2, so it only runs DMA configs.)
# pair 4 is "split": tile 8 signed on Act (bipolar) right after its own
# matmuls (not waiting for tile 9), tile 9 on DVE (binary) — halves the
# end-of-chunk sign-wait before the final mm2.
PAIR_ENGINE = ("act", "dve", "act", "dve", "split")
BINARY_TILES = (2, 3, 6, 7, 9)
MM2_DEFER = 4             # mm1 groups between a pair's last matmul and its mm2

_prog_cache = {}


def _build_program(reps=1, unroll=1, nb_limit=NB):
    # nb_limit < NB builds a truncated-body variant (bench diagnostics only)
    key = ("nc", reps, unroll, nb_limit)
    if key in _prog_cache:
        return _prog_cache[key]

    from contextlib import ExitStack
    import concourse.bacc as bacc
    import concourse.tile as tile
    import concourse.mybir as mybir

    mf8 = mybir.dt.float8e4
    mf32 = mybir.dt.float32
    DR = mybir.MatmulPerfMode.DoubleRow

    # disable_frame_to_traceback keeps source paths out of the BIR so the
    # persistent compile cache is stable across working directories
    nc = bacc.Bacc(
        "TRN2", target_bir_lowering=False, debug=False,
        disable_frame_to_traceback=True,
    )

    st_d = nc.dram_tensor("st", [NB, 128, FG, BCK], mf8, kind="ExternalInput")
    wt_d = nc.dram_tensor("wt", [128, NTL, FG, 128], mf8, kind="ExternalInput")
    cb_d = nc.dram_tensor("cb", [128, NTL, CP], mf8, kind="ExternalInput")
    dots_d = nc.dram_tensor("dots", [C, NB, BCK], mf32, kind="ExternalOutput")

    with tile.TileContext(nc) as tc, ExitStack() as ctx:
        const = ctx.enter_context(tc.tile_pool(name="const", bufs=1))
        stp = ctx.enter_context(tc.tile_pool(name="stp", bufs=3))
        bipp = ctx.enter_context(tc.tile_pool(name="bipp", bufs=4))
        outp = ctx.enter_context(tc.tile_pool(name="outp", bufs=4))
        hvp = ctx.enter_context(tc.tile_pool(name="hvp", bufs=3, space="PSUM"))
        dotsp = ctx.enter_context(tc.tile_pool(name="dotsp", bufs=2, space="PSUM"))

        # ---- preamble: model constants resident across reps ----
        # warm the ScalarE activation table (Sign) while DMAs stream, so
        # the first real sign doesn't eat the 1.3us table load
        warm_in = const.tile([1, 2], mf32, tag="warm_in")
        nc.gpsimd.memset(warm_in[:], 0.0)
        warm_out = const.tile([1, 2], mf8, tag="warm_out")
        nc.scalar.activation(
            warm_out[:], warm_in[:], mybir.ActivationFunctionType.Sign
        )
        # weights: d-tiles 0-4 on SP (needed first), 5-9 on Pool SWDGE —
        # parallel config paths so each lands before its first consumer
        wtA = const.tile([128, NWA, FG, 128], mf8, tag="wtA")
        nc.sync.dma_start(wtA[:], wt_d[:, 0:NWA])
        wtB = const.tile([128, NTL - NWA, FG, 128], mf8, tag="wtB")
        nc.gpsimd.dma_start(wtB[:], wt_d[:, NWA:])
        # centroids via Activation DGE, done before the first mm2
        cb_t = const.tile([128, NTL, CP], mf8, tag="cb")
        nc.scalar.dma_start(cb_t[:], cb_d[:])

        def lhsT(t, u):
            if t < NWA:
                return wtA[:, t, 2 * u : 2 * u + 2, :]
            return wtB[:, t - NWA, 2 * u : 2 * u + 2, :]

        def body():
            # per-inference: samples in (8 chunk DMAs on SP, triple-
            # buffered prefetch), compute, dots out
            st_ts = []
            for j in range(nb_limit):
                st_t = stp.tile([128, FG, BCK], mf8, tag="st_t")
                if j == 0:
                    # chunk 0 is head-exposed after the For_i barrier:
                    # split its DMA across the SP and Act config queues so
                    # both halves transfer in parallel (~0.8us shorter head)
                    nc.sync.dma_start(
                        st_t[:, : FG // 2, :], st_d[j][:, : FG // 2]
                    )
                    nc.scalar.dma_start(
                        st_t[:, FG // 2 :, :], st_d[j][:, FG // 2 :]
                    )
                else:
                    nc.sync.dma_start(st_t[:], st_d[j])
                st_ts.append(st_t)

            # software-pipelined PE stream: each pair's mm2 is scheduled
            # MM2_DEFER mm1-groups after its last sign was issued, so the
            # in-order PE never waits on the sign engines.  The schedule
            # is built as data first and emitted in one lexical scope
            # (the tile pools infer allocation scope from the call stack).
            sched = []
            pending = []          # (due_group, ("mm2"/"sign", j, t))
            gi = 0
            for j in range(nb_limit):
                for t in range(NTL):
                    sched.append(("mm1", j, t))
                    gi += 1
                    while pending and pending[0][0] <= gi:
                        sched.append(pending.pop(0)[1])
                    if t % 2 == 0 and PAIR_ENGINE[t // 2] == "split":
                        sched.append(("sign0", j, t))
                    if t % 2 == 1:
                        sched.append(("sign", j, t))
                        pending.append((gi + MM2_DEFER, ("mm2", j, t)))
            while pending:
                sched.append(pending.pop(0)[1])

            pds = {}
            phs = {}
            bips = {}
            for op, j, t in sched:
                if op == "mm1":
                    if t == 0:
                        pd = dotsp.tile([CP, BCK], mf32, tag="pd")
                        pds[j] = pd
                    if t % 2 == 0:
                        ph = hvp.tile([128, 2, BCK], mf32, tag="ph")
                        phs[(j, t // 2)] = ph
                    else:
                        ph = phs[(j, t // 2)]
                    for u in range(FG // 2):
                        nc.tensor.matmul(
                            ph[:, t % 2, :],
                            lhsT=lhsT(t, u),
                            rhs=st_ts[j][:, 2 * u : 2 * u + 2, :],
                            start=(u == 0), stop=(u == FG // 2 - 1),
                            perf_mode=DR,
                        )
                elif op == "sign0":
                    # split pair: Act signs tile t while tile t+1's matmuls
                    # still run (disjoint bank of the same PSUM pair tile)
                    bip2 = bipp.tile([128, 2, BCK], mf8, tag="bip2")
                    bips[(j, t // 2)] = bip2
                    nc.scalar.activation(
                        bip2[:, 0, :], phs[(j, t // 2)][:, 0, :],
                        mybir.ActivationFunctionType.Sign,
                    )
                elif op == "sign":
                    eng = PAIR_ENGINE[t // 2]
                    ph = phs.pop((j, t // 2))
                    if eng == "split":
                        # slot 1 on DVE (binary); slot 0 already signed
                        bip2 = bips[(j, t // 2)]
                        nc.vector.tensor_scalar(
                            bip2[:, 1, :], ph[:, 1, :], 0.0, None,
                            mybir.AluOpType.is_gt,
                        )
                    else:
                        # one instruction binarizes the whole 2-bank pair
                        bip2 = bipp.tile([128, 2, BCK], mf8, tag="bip2")
                        bips[(j, t // 2)] = bip2
                        if eng == "act":
                            nc.scalar.activation(
                                bip2[:], ph[:],
                                mybir.ActivationFunctionType.Sign,
                            )
                        else:
                            nc.vector.tensor_scalar(
                                bip2[:], ph[:], 0.0, None,
                                mybir.AluOpType.is_gt,
                            )
                else:
                    bip2 = bips.pop((j, t // 2))
                    nc.tensor.matmul(
                        pds[j][:], lhsT=cb_t[:, t - 1 : t + 1, :],
                        rhs=bip2[:],
                        start=(t == 1), stop=(t == NTL - 1),
                        perf_mode=DR,
                    )
                    if t == NTL - 1:
                        # dots out: DVE copies PSUM->SBUF, Pool SWDGE
                        # configures the DRAM write from SBUF (legal for
                        # GPSIMD).  Halved so the second copy overlaps the
                        # first DMA — trims the tail before the barrier.
                        out_t = outp.tile([C, BCK], mf32, tag="out_t")
                        h = BCK // 2
                        nc.vector.tensor_copy(
                            out_t[:, :h], pds[j][:C, :h]
                        )
                        nc.gpsimd.dma_start(
                            dots_d[:, j, :h], out_t[:, :h]
                        )
                        nc.vector.tensor_copy(
                            out_t[:, h:], pds[j][:C, h:]
                        )
                        nc.gpsimd.dma_start(
                            dots_d[:, j, h:], out_t[:, h:]
                        )

        if reps == 1 and unroll == 1:
            body()
        else:
            # benchmarking only: repeat the full per-inference body in a HW
            # loop so device time can be extracted as a wall-clock
            # differential.  For_i drains all engines at every iteration
            # (InstAllEngineBarrier in its reset block), so `unroll` bodies
            # are emitted per iteration — back-to-back bodies pipeline
            # freely, which is the true steady state.
            with tc.For_i(0, reps, 1):
                for _ in range(unroll):
                    body()

    nc.compile()
    # Rewrite source-location debug info to constants so the serialized BIR
    # (and therefore the persistent compile-cache key) is independent of
    # file paths and call sites.
    def _neutral(d):
        if d is None or not hasattr(d, "filename"):
            return d
        return type(d)(
            op_name=d.op_name, tensorizer_id=d.tensorizer_id,
            filename="kernel.py", lineno=0,
            bass_funcname=d.bass_funcname, kernel_name=d.kernel_name,
            ant_traceback=None, ant_layer=d.ant_layer,
            ant_annotation=d.ant_annotation,
        )

    for fn in nc.m.functions:
        for blk in fn.blocks:
            for inst in blk.instructions:
                if inst.debug is not None:
                    inst.debug = _neutral(inst.debug)
        for alloc in fn.allocations:
            for ml in getattr(alloc, "memorylocations", None) or []:
                if getattr(ml, "ant_debug", None) is not None:
                    ml.ant_debug = _neutral(ml.ant_debug)
    _prog_cache[key] = nc
    return nc


def _binary_mask_d():
    # global d indices whose local tile uses the binary (is_gt) path
    tl = (np.arange(D) // 128) % NTL
    return np.isin(tl, BINARY_TILES)


def _pack_w(W):
    # W^T padded [F, DPAD], partition-major per d-tile:
    # packed[p, t, g, j] = W^T[g*128+p, t*128+j]
    WT = np.zeros((F, DPAD), dtype=f8)
    WT[:, :D] = W.astype(f8).T
    return np.ascontiguousarray(
        WT.reshape(FG, 128, NT, 128).transpose(1, 2, 0, 3)
    )


def _pack_cb(centroids):
    # bipolar centroids^T padded [DPAD, CP], doubled on binary tiles:
    # packed[p, t, c] = scale[d] * cb^T[t*128+p, c]
    cbb = 2.0 * centroids.astype(np.float32) - 1.0        # [C, D]
    scale = np.where(_binary_mask_d(), 2.0, 1.0)
    cbT = np.zeros((DPAD, CP), dtype=np.float32)
    cbT[:D, :C] = (cbb * scale[None, :]).T
    cbT = cbT.astype(f8)
    return np.ascontiguousarray(cbT.reshape(NT, 128, CP).transpose(1, 0, 2))


def _pack_st(x8):
    # full samples fp8 [B, F] -> packed[j, p, g, b] = x8[j*BCK+b, g*128+p]
    return np.ascontiguousarray(
        x8.T.reshape(FG, 128, NB, BCK).transpose(2, 1, 0, 3)
    )


def make_in_maps(inputs):
    samples = np.asarray(inputs["samples"], dtype=np.float32)
    W = np.asarray(inputs["W"], dtype=np.float32)
    centroids = np.asarray(inputs["centroids"], dtype=np.float32)
    assert samples.shape == (B, F) and W.shape == (D, F) and centroids.shape == (C, D)

    x8 = (samples - 0.5).astype(f8)
    st_packed = _pack_st(x8)
    wt_packed = _pack_w(W)                                # [128, NT, FG, 128]
    cb_packed = _pack_cb(centroids)                       # [128, NT, CP]

    in_maps = []
    for i in range(NCORES):
        in_maps.append(
            {
                "st": st_packed,
                "wt": np.ascontiguousarray(
                    wt_packed[:, i * NTL : (i + 1) * NTL]
                ),
                "cb": np.ascontiguousarray(
                    cb_packed[:, i * NTL : (i + 1) * NTL, :]
                ),
            }
        )
    return in_maps


def _postprocess(results, centroids):
    # partial dots: sum over cores (exact integers in fp32), undo the
    # binary-tile doubling, affine to similarity counts
    dots = np.zeros((C, NB, BCK), dtype=np.float64)
    for r in results:
        dots += np.asarray(r["dots"], dtype=np.float64)
    dots = dots.reshape(C, B)
    cbb = 2.0 * np.asarray(centroids, dtype=np.float64) - 1.0
    binsum = cbb[:, _binary_mask_d()].sum(axis=1)         # [C]
    sim = np.rint(0.5 * (np.float64(D) + dots - binsum[:, None]))
    return sim.T.astype(np.int32)


def _enable_jax_compile_cache():
    # Persistent XLA/NEFF compile cache so repeated invocations (fresh
    # processes included) skip the multi-minute neuronx-cc compile.
    try:
        import jax

        d = os.path.expanduser("~/.cache/trn_knn_kernel_jax_cache")
        os.makedirs(d, exist_ok=True)
        jax.config.update("jax_compilation_cache_dir", d)
        jax.config.update("jax_persistent_cache_min_entry_size_bytes", 0)
        jax.config.update("jax_persistent_cache_min_compile_time_secs", 0)
    except Exception:
        pass


def _run(inputs, trace=False, reps=1):
    _enable_jax_compile_cache()
    from concourse.bass_utils import run_bass_kernel_spmd

    in_maps = make_in_maps(inputs)
    nc = _build_program(reps=reps)
    res = run_bass_kernel_spmd(nc, in_maps, list(range(NCORES)), trace=trace)
    out = _postprocess(res.results, inputs["centroids"])
    return out, res


def kernel(samples, W, centroids):
    out, _ = _run({"samples": samples, "W": W, "centroids": centroids})
    return out


# revision 32
# speedup vs baseline: 1.0239x; 1.0239x over previous
"""Trainium2 Bass kernel for nn_Classifier_22299470201420 (retrieval_knn).

Reference computation:
    hv   = (samples - 0.5) @ W.T          # [B, D] random projection
    bip  = where(hv > 0, 1, -1)           # bipolar hypervector
    dots = bip @ (2*centroids - 1).T      # [B, C] bipolar dot products
    sim  = int32(0.5 * (D + dots))        # hamming similarity counts

Sharding: tensor-parallel over the D (dimensions) axis — D is zero-padded
10000 -> 10240 = 80 d-tiles of 128; each of the 8 cores owns 10 d-tiles
and the FULL batch, producing a partial dots [C, B] that the host sums
(exact integer adds, no on-device collective needed).

Precision: samples-0.5 is quantized to fp8e4m3 only (no lo-residual
pass).  W is {-1,+1} — exact in fp8 — and hv accumulates in fp32 PSUM,
so the only error is the fp8 input quantization, which flips the sign
of hv for ~0.8% of bits (only where |hv| is tiny): measured host-side
rel err 1.73e-3 on the output counts, far below the 2e-2 gate.

Device kernel (per core), all work inside the timed body:
  - 8 b-chunks of 512 samples; per chunk 10 d-tiles of 4 fp8 DoubleRow
    matmuls (K=256 each) accumulate hv^T into PSUM.  The PE is the
    critical resource (HW-measured ~274ns per 512-row DR matmul,
    weight loads fully hidden); everything else is sized to stay under
    it so the PE never stalls.
  - binarize hv^T: one instruction per d-tile pair over a 2-bank PSUM
    tile, split across ScalarE (Sign -> bipolar) and DVE (is_gt ->
    binary {1,0}); binary tiles use doubled centroids (exact fp8) in
    matmul2 and the host subtracts the per-class sum of cb over binary
    tiles — exact.  Each mm2 is deferred 4 mm1-groups in the PE stream
    so the in-order PE never waits on the sign engines.
  - matmul2 runs fp8 DoubleRow over d-tile pairs, accumulating into one
    PSUM bank [112, 512] per chunk; DVE copies it to SBUF and a Pool
    SWDGE DMA writes it out (GPSIMD cannot read PSUM on real TRN2).
  - head-latency: input DMA configs are spread over SP/Pool/Act so no
    single sequencer serializes the head, and a dummy 1-element Sign
    preloads the ScalarE activation table (1.3us) during the DMA head
    instead of stalling the first mm2.
  - The final affine 0.5*(D + dots - binsum) + int32 cast + transpose
    happens on the host on the [C, B] summed partials.
"""

import os

import numpy as np
import ml_dtypes

B, F, D, C = 4096, 1024, 10000, 100
NCORES = 8
NT = 80                   # global 128-wide d-tiles (D padded 10000 -> 10240)
NTL = NT // NCORES        # d-tiles per core
DPAD = NT * 128           # 10240
FG = F // 128             # 8 f-groups of 128
NB = 8                    # b-chunks per core (full batch)
BCK = B // NB             # 512
CP = 112                  # C padded so fp8 DoubleRow weight strides are 16B-aligned
NWA = 5                   # d-tiles in the first weight DMA (needed earliest)

bf16 = ml_dtypes.bfloat16
f8 = ml_dtypes.float8_e4m3

# engine per local d-tile pair: Act pairs produce bipolar {-1,+1} via
# Sign; DVE pairs produce binary {1,0} via is_gt and use doubled
# centroids with an exact host-side correction.  Each pair is binarized
# by ONE instruction over a 2-bank PSUM tile [128, 2, 512].  (Pool/
# GPSIMD cannot read PSUM on real TRN2, so it only runs DMA configs.)
PAIR_ENGINE = ("act", "dve", "act", "dve", "act")
BINARY_TILES = tuple(
    t for t in range(NTL) if PAIR_ENGINE[t // 2] != "act"
)
MM2_DEFER = 4             # mm1 groups between a pair's last matmul and its mm2

_prog_cache = {}


def _build_program(reps=1, unroll=1, nb_limit=NB):
    # nb_limit < NB builds a truncated-body variant (bench diagnostics only)
    key = ("nc", reps, unroll, nb_limit)
    if key in _prog_cache:
        return _prog_cache[key]

    from contextlib import ExitStack
    import concourse.bacc as bacc
    import concourse.tile as tile
    import concourse.mybir as mybir

    mf8 = mybir.dt.float8e4
    mf32 = mybir.dt.float32
    DR = mybir.MatmulPerfMode.DoubleRow

    # disable_frame_to_traceback keeps source paths out of the BIR so the
    # persistent compile cache is stable across working directories
    nc = bacc.Bacc(
        "TRN2", target_bir_lowering=False, debug=False,
        disable_frame_to_traceback=True,
    )

    st_d = nc.dram_tensor("st", [NB, 128, FG, BCK], mf8, kind="ExternalInput")
    wt_d = nc.dram_tensor("wt", [128, NTL, FG, 128], mf8, kind="ExternalInput")
    cb_d = nc.dram_tensor("cb", [128, NTL, CP], mf8, kind="ExternalInput")
    dots_d = nc.dram_tensor("dots", [C, NB, BCK], mf32, kind="ExternalOutput")

    with tile.TileContext(nc) as tc, ExitStack() as ctx:
        const = ctx.enter_context(tc.tile_pool(name="const", bufs=1))
        stp = ctx.enter_context(tc.tile_pool(name="stp", bufs=3))
        bipp = ctx.enter_context(tc.tile_pool(name="bipp", bufs=4))
        outp = ctx.enter_context(tc.tile_pool(name="outp", bufs=4))
        hvp = ctx.enter_context(tc.tile_pool(name="hvp", bufs=3, space="PSUM"))
        dotsp = ctx.enter_context(tc.tile_pool(name="dotsp", bufs=2, space="PSUM"))

        # ---- preamble: model constants resident across reps ----
        # warm the ScalarE activation table (Sign) while DMAs stream, so
        # the first real sign doesn't eat the 1.3us table load
        warm_in = const.tile([1, 2], mf32, tag="warm_in")
        nc.gpsimd.memset(warm_in[:], 0.0)
        warm_out = const.tile([1, 2], mf8, tag="warm_out")
        nc.scalar.activation(
            warm_out[:], warm_in[:], mybir.ActivationFunctionType.Sign
        )
        # weights: d-tiles 0-4 on SP (needed first), 5-9 on Pool SWDGE —
        # parallel config paths so each lands before its first consumer
        wtA = const.tile([128, NWA, FG, 128], mf8, tag="wtA")
        nc.sync.dma_start(wtA[:], wt_d[:, 0:NWA])
        wtB = const.tile([128, NTL - NWA, FG, 128], mf8, tag="wtB")
        nc.gpsimd.dma_start(wtB[:], wt_d[:, NWA:])
        # centroids via Activation DGE, done before the first mm2
        cb_t = const.tile([128, NTL, CP], mf8, tag="cb")
        nc.scalar.dma_start(cb_t[:], cb_d[:])

        def lhsT(t, u):
            if t < NWA:
                return wtA[:, t, 2 * u : 2 * u + 2, :]
            return wtB[:, t - NWA, 2 * u : 2 * u + 2, :]

        def body():
            # per-inference: samples in (8 chunk DMAs on SP, triple-
            # buffered prefetch), compute, dots out
            st_ts = []
            for j in range(nb_limit):
                st_t = stp.tile([128, FG, BCK], mf8, tag="st_t")
                if j == 0:
                    # chunk 0 is head-exposed after the For_i barrier:
                    # split its DMA across the SP and Act config queues so
                    # both halves transfer in parallel (~0.8us shorter head)
                    nc.sync.dma_start(
                        st_t[:, : FG // 2, :], st_d[j][:, : FG // 2]
                    )
                    nc.scalar.dma_start(
                        st_t[:, FG // 2 :, :], st_d[j][:, FG // 2 :]
                    )
                else:
                    nc.sync.dma_start(st_t[:], st_d[j])
                st_ts.append(st_t)

            # software-pipelined PE stream: each pair's mm2 is scheduled
            # MM2_DEFER mm1-groups after its last sign was issued, so the
            # in-order PE never waits on the sign engines.  The schedule
            # is built as data first and emitted in one lexical scope
            # (the tile pools infer allocation scope from the call stack).
            sched = []
            pending = []          # (due_group, ("mm2"/"sign", j, t))
            gi = 0
            for j in range(nb_limit):
                for t in range(NTL):
                    sched.append(("mm1", j, t))
                    gi += 1
                    while pending and pending[0][0] <= gi:
                        sched.append(pending.pop(0)[1])
                    if t % 2 == 1:
                        sched.append(("sign", j, t))
                        pending.append((gi + MM2_DEFER, ("mm2", j, t)))
            while pending:
                sched.append(pending.pop(0)[1])

            pds = {}
            phs = {}
            bips = {}
            for op, j, t in sched:
                if op == "mm1":
                    if t == 0:
                        pd = dotsp.tile([CP, BCK], mf32, tag="pd")
                        pds[j] = pd
                    if t % 2 == 0:
                        ph = hvp.tile([128, 2, BCK], mf32, tag="ph")
                        phs[(j, t // 2)] = ph
                    else:
                        ph = phs[(j, t // 2)]
                    for u in range(FG // 2):
                        nc.tensor.matmul(
                            ph[:, t % 2, :],
                            lhsT=lhsT(t, u),
                            rhs=st_ts[j][:, 2 * u : 2 * u + 2, :],
                            start=(u == 0), stop=(u == FG // 2 - 1),
                            perf_mode=DR,
                        )
                elif op == "sign":
                    # one instruction binarizes the whole 2-bank pair
                    bip2 = bipp.tile([128, 2, BCK], mf8, tag="bip2")
                    bips[(j, t // 2)] = bip2
                    ph = phs.pop((j, t // 2))
                    if PAIR_ENGINE[t // 2] == "act":
                        nc.scalar.activation(
                            bip2[:], ph[:],
                            mybir.ActivationFunctionType.Sign,
                        )
                    else:
                        nc.vector.tensor_scalar(
                            bip2[:], ph[:], 0.0, None,
                            mybir.AluOpType.is_gt,
                        )
                else:
                    bip2 = bips.pop((j, t // 2))
                    nc.tensor.matmul(
                        pds[j][:], lhsT=cb_t[:, t - 1 : t + 1, :],
                        rhs=bip2[:],
                        start=(t == 1), stop=(t == NTL - 1),
                        perf_mode=DR,
                    )
                    if t == NTL - 1:
                        # dots out: DVE copies PSUM->SBUF, Pool SWDGE
                        # configures the DRAM write from SBUF (legal for
                        # GPSIMD).  Halved so the second copy overlaps the
                        # first DMA — trims the tail before the barrier.
                        out_t = outp.tile([C, BCK], mf32, tag="out_t")
                        h = BCK // 2
                        nc.vector.tensor_copy(
                            out_t[:, :h], pds[j][:C, :h]
                        )
                        nc.gpsimd.dma_start(
                            dots_d[:, j, :h], out_t[:, :h]
                        )
                        nc.vector.tensor_copy(
                            out_t[:, h:], pds[j][:C, h:]
                        )
                        nc.gpsimd.dma_start(
                            dots_d[:, j, h:], out_t[:, h:]
                        )

        if reps == 1 and unroll == 1:
            body()
        else:
            # benchmarking only: repeat the full per-inference body in a HW
            # loop so device time can be extracted as a wall-clock
            # differential.  For_i drains all engines at every iteration
            # (InstAllEngineBarrier in its reset block), so `unroll` bodies
            # are emitted per iteration — back-to-back bodies pipeline
            # freely, which is the true steady state.
            with tc.For_i(0, reps, 1):
                for _ in range(unroll):
                    body()

    nc.compile()
    # Rewrite source-location debug info to constants so the serialized BIR
    # (and therefore the persistent compile-cache key) is independent of
    # file paths and call sites.
    def _neutral(d):
        if d is None or not hasattr(d, "filename"):
            return d
        return type(d)(
            op_name=d.op_name, tensorizer_id=d.tensorizer_id,
            filename="kernel.py", lineno=0,
            bass_funcname=d.bass_funcname, kernel_name=d.kernel_name,
            ant_traceback=None, ant_layer=d.ant_layer,
            ant_annotation=d.ant_annotation,
        )

    for fn in nc.m.functions:
        for blk in fn.blocks:
            for inst in blk.instructions:
                if inst.debug is not None:
                    inst.debug = _neutral(inst.debug)
        for alloc in fn.allocations:
            for ml in getattr(alloc, "memorylocations", None) or []:
                if getattr(ml, "ant_debug", None) is not None:
                    ml.ant_debug = _neutral(ml.ant_debug)
    _prog_cache[key] = nc
    return nc


def _binary_mask_d():
    # global d indices whose local tile uses the binary (is_gt) path
    tl = (np.arange(D) // 128) % NTL
    return np.isin(tl, BINARY_TILES)


def _pack_w(W):
    # W^T padded [F, DPAD], partition-major per d-tile:
    # packed[p, t, g, j] = W^T[g*128+p, t*128+j]
    WT = np.zeros((F, DPAD), dtype=f8)
    WT[:, :D] = W.astype(f8).T
    return np.ascontiguousarray(
        WT.reshape(FG, 128, NT, 128).transpose(1, 2, 0, 3)
    )


def _pack_cb(centroids):
    # bipolar centroids^T padded [DPAD, CP], doubled on binary tiles:
    # packed[p, t, c] = scale[d] * cb^T[t*128+p, c]
    cbb = 2.0 * centroids.astype(np.float32) - 1.0        # [C, D]
    scale = np.where(_binary_mask_d(), 2.0, 1.0)
    cbT = np.zeros((DPAD, CP), dtype=np.float32)
    cbT[:D, :C] = (cbb * scale[None, :]).T
    cbT = cbT.astype(f8)
    return np.ascontiguousarray(cbT.reshape(NT, 128, CP).transpose(1, 0, 2))


def _pack_st(x8):
    # full samples fp8 [B, F] -> packed[j, p, g, b] = x8[j*BCK+b, g*128+p]
    return np.ascontiguousarray(
        x8.T.reshape(FG, 128, NB, BCK).transpose(2, 1, 0, 3)
    )


def make_in_maps(inputs):
    samples = np.asarray(inputs["samples"], dtype=np.float32)
    W = np.asarray(inputs["W"], dtype=np.float32)
    centroids = np.asarray(inputs["centroids"], dtype=np.float32)
    assert samples.shape == (B, F) and W.shape == (D, F) and centroids.shape == (C, D)

    x8 = (samples - 0.5).astype(f8)
    st_packed = _pack_st(x8)
    wt_packed = _pack_w(W)                                # [128, NT, FG, 128]
    cb_packed = _pack_cb(centroids)                       # [128, NT, CP]

    in_maps = []
    for i in range(NCORES):
        in_maps.append(
            {
                "st": st_packed,
                "wt": np.ascontiguousarray(
                    wt_packed[:, i * NTL : (i + 1) * NTL]
                ),
                "cb": np.ascontiguousarray(
                    cb_packed[:, i * NTL : (i + 1) * NTL, :]
                ),
            }
        )
    return in_maps


def _postprocess(results, centroids):
    # partial dots: sum over cores (exact integers in fp32), undo the
    # binary-tile doubling, affine to similarity counts
    dots = np.zeros((C, NB, BCK), dtype=np.float64)
    for r in results:
        dots += np.asarray(r["dots"], dtype=np.float64)
    dots = dots.reshape(C, B)
    cbb = 2.0 * np.asarray(centroids, dtype=np.float64) - 1.0
    binsum = cbb[:, _binary_mask_d()].sum(axis=1)         # [C]
    sim = np.rint(0.5 * (np.float64(D) + dots - binsum[:, None]))
    return sim.T.astype(np.int32)


def _enable_jax_compile_cache():
    # Persistent XLA/NEFF compile cache so repeated invocations (fresh
    # processes included) skip the multi-minute neuronx-cc compile.
    try:
        import jax

        d = os.path.expanduser("~/.cache/trn_knn_kernel_jax_cache")
        os.makedirs(d, exist_ok=True)
        jax.config.update("jax_compilation_cache_dir", d)
        jax.config.update("jax_persistent_cache_min_entry_size_bytes", 0)
        jax.config.update("jax_persistent_cache_min_compile_time_secs", 0)
    except Exception:
        pass


def _run(inputs, trace=False, reps=1):
    _enable_jax_compile_cache()
    from concourse.bass_utils import run_bass_kernel_spmd

    in_maps = make_in_maps(inputs)
    nc = _build_program(reps=reps)
    res = run_bass_kernel_spmd(nc, in_maps, list(range(NCORES)), trace=trace)
    out = _postprocess(res.results, inputs["centroids"])
    return out, res


def kernel(samples, W, centroids):
    out, _ = _run({"samples": samples, "W": W, "centroids": centroids})
    return out
